# revision 48
# baseline (speedup 1.0000x reference)
"""Sharded causal multi-head attention for 8 Trainium2 NeuronCores.

kernel(**inputs) takes the FULL inputs (Q, K, V, mask, Wq, bq, Wk, bk,
Wv, bv, Wo, bo) and returns the FULL [2, 2048, 1024] float32 output.

Sharding (data + head/tensor parallel): core c = 4*b + g handles batch
b in {0,1} and head-group g in {0..3} (4 heads, 256 dims). W_q/W_k/W_v
are column-parallel, W_o row-parallel; the host sums the 4 per-batch
row-parallel partials and adds (bo + bv @ Wo.T) - the v-bias commutes
out of the softmax-weighted sum because prob rows sum to 1.

v5 structure (2-pass global software pipeline):
  - ScalarE exp is the irreducible pacer (~89us of exp tiles); all other
    work is scheduled around keeping it fed.
  - Each query-block window runs TWICE (one head-pair per pass), so the
    av accumulator needs only 2 PSUM banks. That frees a dedicated
    2-bank "aux" slot for every projection (k/q chains, v, out-proj),
    leaving the 2-slot score rotation exclusively ACT-drained - the PE
    score stream is never stalled behind a DVE projection evacuation.
  - k/q projections for quarter Q+1 stream through window Q as
    fine-grained filler thunks; out-projections are deferred to the
    ACT-bound later windows; v fills in its own window.
  - x DMAs are throttled by pool-slot reuse (xq/xk bufs=6, xv bufs=4)
    so prefetch never steals HBM bandwidth from the critical quarter.
  - causal masks are generated on device (GpSimd affine_select into
    dm4 at startup) instead of a 1MB host DMA.
  - pass/window boundaries pre-emit the next pass's first score steps
    so ACT runs through the trailing avs + normalize; a 10-matmul
    warmup unthrottles the PE clock (HAM) before the first projections.
"""

import json
import sys

for _p in ("/opt/trn_rl_repo", "/opt/trn_rl_repo/concourse"):
    if _p not in sys.path:
        sys.path.insert(0, _p)

import numpy as np

import bass_rust
import concourse.bass as bass
import concourse.mybir as mybir
import concourse.tile as tile
from concourse import bass_utils
from concourse.bass import ts
from concourse.vector_clock import ScopedClock

F32 = mybir.dt.float32
F16 = mybir.dt.float16  # 10-bit mantissa; every intermediate is O(1)-bounded
S = 2048
D = 1024
HG = 256  # head-group dims (4 heads x 64)
NH = 4  # heads per core
KC = D // 128
NQB = 4
QB = 512
NSC = S // 128

# --------------------------------------------------------------------------
# Environment patches: this container's walrus accepts only ONE sync-wait
# command per instruction, but Tile emits several (and its epilogue drain
# carries one per outstanding proc sem). Split extras onto single-wait NoOps.
# --------------------------------------------------------------------------

_patched = False


def _drain_and_barrier_split(self, tick_clock, wait_clock):
    nc = self.nc
    probe = nc.sync.nop()
    wait_clock.add_sem_waits(probe.ins, ScopedClock({None: tick_clock.global_clock}))
    si = probe.ins.sync_info
    waits = list(si.on_wait) if si is not None and si.on_wait else []
    if len(waits) > 1:
        si.on_wait = [waits[0]]
        for w in waits[1:]:
            nop = nc.sync.nop()
            nop.ins.sync_info = bass_rust.SyncInfo(on_wait=[w], on_update=[])
    nc.sync.drain()
    nc.all_engine_barrier()
    assert self.sems is not None
    popped = nc._tile_sem_poison_stack.pop()
    assert popped is self._sem_poison
    nc.clear_and_free_semaphores(list(self.sems.allocated().values()))
    nc.all_engine_barrier()


def _split_waits_json(raw):
    j = json.loads(raw)
    changed = False
    for f in j.get("functions", []):
        for bb in f.get("blocks", []):
            out = []
            for inst in bb.get("instructions", []):
                si = inst.get("sync_info")
                waits = (si or {}).get("on_wait") or []
                if len(waits) > 1:
                    for k, w in enumerate(waits[:-1]):
                        nop = {
                            "engine": inst["engine"],
                            "ins": [],
                            "name": f"{inst['name']}-ws{k}",
                            "opcode": "NoOp",
                            "outs": [],
                            "sync_info": {"on_update": [], "on_wait": [w]},
                        }
                        if "debug" in inst:
                            nop["debug"] = inst["debug"]
                        out.append(nop)
                    si["on_wait"] = [waits[-1]]
                    changed = True
                out.append(inst)
            if changed:
                bb["instructions"] = out
    return json.dumps(j).encode() if changed else raw


def _apply_patches():
    global _patched
    if _patched:
        return
    tile.TileContext._drain_and_barrier = _drain_and_barrier_split
    orig_to_json = bass.Bass.to_json_bytes
    bass.Bass.to_json_bytes = lambda self: _split_waits_json(orig_to_json(self))
    # NOTE: do NOT enable walrus ldw-opt here - it crashes codegen
    # (visitInstLdweights) for 2-byte matmul dtypes.
    _patched = True


# --------------------------------------------------------------------------
# Per-core Bass program
# --------------------------------------------------------------------------


def _build():
    nc = bass.Bass("TRN2", target_bir_lowering=False, debug=False, num_devices=8)

    # all host-packed: partition dim first, fully contiguous per partition
    xqT = nc.dram_tensor("xqT", [128, NQB, KC, QB], F16, kind="ExternalInput").ap()
    xkT = nc.dram_tensor("xkT", [128, NQB, KC, QB], F16, kind="ExternalInput").ap()
    xvT = nc.dram_tensor("xvT", [128, NQB, KC, QB], F16, kind="ExternalInput").ap()
    wqT = nc.dram_tensor("wqT", [128, KC, HG], F16, kind="ExternalInput").ap()
    wkT = nc.dram_tensor("wkT", [128, KC, HG], F16, kind="ExternalInput").ap()
    wvT = nc.dram_tensor("wvT", [128, KC, HG], F16, kind="ExternalInput").ap()
    woT = nc.dram_tensor("woT", [128, 2, D], F16, kind="ExternalInput").ap()
    bq_d = nc.dram_tensor("bq", [128, 2], F32, kind="ExternalInput").ap()
    bk_d = nc.dram_tensor("bk", [128, 2], F32, kind="ExternalInput").ap()
    out_d = nc.dram_tensor("out", [S, D], F16, kind="ExternalOutput").ap()

    from contextlib import ExitStack

    with tile.TileContext(nc) as tc, ExitStack() as ctx:
        consts = ctx.enter_context(tc.tile_pool(name="consts", bufs=1))
        qkv_sb = ctx.enter_context(tc.tile_pool(name="qkv", bufs=1))
        # xq/xk at 6 bufs (1.5 quarters): a later quarter's chunk DMA waits
        # (via slot reuse) until the matching earlier chunk has been read,
        # so prefetch never steals HBM bandwidth from the critical quarter
        xk_pool = ctx.enter_context(tc.tile_pool(name="xk", bufs=6))
        xq_pool = ctx.enter_context(tc.tile_pool(name="xq", bufs=6))
        xv_pool = ctx.enter_context(tc.tile_pool(name="xv", bufs=4))
        et_pool = ctx.enter_context(tc.tile_pool(name="et", bufs=20))
        small = ctx.enter_context(tc.tile_pool(name="small", bufs=2))
        outsb = ctx.enter_context(tc.tile_pool(name="outsb", bufs=3))

        # PSUM (8 banks): 2 score slots (2 banks each, ACT-drained only) +
        # one aux slot (2 banks: kq/v/out projections, DVE-drained) + one
        # av pair-accumulator slot (2 banks)
        ps_sc = ctx.enter_context(tc.tile_pool(name="ps_sc", bufs=2, space="PSUM"))
        ps_aux = ctx.enter_context(tc.tile_pool(name="ps_aux", bufs=1, space="PSUM"))
        ps_av = ctx.enter_context(tc.tile_pool(name="ps_av", bufs=1, space="PSUM"))

        # ---- DMA helpers: chunks are >=2KB contiguous per partition ----
        def x_quarter(pool, dram, quarter, name):
            chunks = []
            for c4 in range(4):
                t = pool.tile([128, 2, QB], F16, name=name)
                nc.sync.dma_start(t[:], dram[:, quarter, 2 * c4 : 2 * c4 + 2, :])
                chunks.append(t)
            return chunks

        def xs(chunks, kc):
            return chunks[kc // 2][:, kc % 2, :]

        def w_load_interleaved(name, dram, xpool, xdram, xname):
            # wc0, xc0, xc1, wc1, xc2, xc3: first matmul needs only wc0+xc0
            wchunks, xchunks = [], []
            wt = consts.tile([128, 4, HG], F16, name=f"{name}0")
            nc.sync.dma_start(wt[:], dram[:, 0:4, :])
            wchunks.append(wt)
            for c4 in range(4):
                t = xpool.tile([128, 2, QB], F16, name=xname)
                nc.sync.dma_start(t[:], xdram[:, 0, 2 * c4 : 2 * c4 + 2, :])
                xchunks.append(t)
                if c4 == 1:
                    wt = consts.tile([128, 4, HG], F16, name=f"{name}1")
                    nc.sync.dma_start(wt[:], dram[:, 4:8, :])
                    wchunks.append(wt)
            return wchunks, xchunks

        # ---- prologue DMAs: quarter-0 k/q first (they gate everything) ----
        w_sb = {}
        xk_t, xq_t, xv_t = {}, {}, {}
        w_sb["wk"], xk_t[0] = w_load_interleaved("wkt", wkT, xk_pool, xkT, "xkq")
        bk_sb = consts.tile([128, 2], F32, name="bkt")
        nc.sync.dma_start(bk_sb[:], bk_d[:])
        w_sb["wq"], xq_t[0] = w_load_interleaved("wqt", wqT, xq_pool, xqT, "xqq")
        bq_sb = consts.tile([128, 2], F32, name="bqt")
        nc.sync.dma_start(bq_sb[:], bq_d[:])

        def ws(name, kc):
            return w_sb[name][kc // 4][:, kc % 4, :]

        # ACT table warmup: load the natural_log_exp set before it matters
        warm = consts.tile([128, 8], F32, name="warm")
        nc.vector.memset(warm[:], 1.0)
        warm2 = consts.tile([128, 8], F16, name="warm2")
        nc.scalar.activation(warm2[:], warm[:], mybir.ActivationFunctionType.Exp)

        # PE clock warmup: the HAM clock-gate holds the PE at 1.2GHz until
        # it has seen a ~3.4us busy window. The prologue projections land
        # right when the first DMAs complete; without this they all run at
        # half clock. ~10 dummy matmuls (~5us cold) warm the array while
        # the first x/w chunks are still in flight.
        wsc = consts.tile([128, QB], F16, name="wsc")
        nc.vector.memset(wsc[:], 0.125)
        warm_ps = ps_aux.tile([128, 2, QB], F32, name="aux")
        for r in range(10):
            nc.tensor.matmul(
                warm_ps[:, 0, :],
                wsc[:, 0:128],
                wsc[:],
                start=(r == 0),
                stop=(r == 9),
            )
        nc.vector.tensor_copy(wsc[:], warm_ps[:, 0, :])

        # ---- persistent activations ----
        q_pad = [qkv_sb.tile([128, S], F16, name=f"qp{h}") for h in range(NH)]
        kT_sb = qkv_sb.tile([128, 2, S], F16, name="kT")
        v_sb = qkv_sb.tile([128, NSC, NH * 128], F16, name="vp")
        attnT_sb = qkv_sb.tile([128, 2, S], F16, name="attnT")

        v_view = v_sb.rearrange("p c (h x) -> p c h x", x=128)
        nc.vector.memset(v_view[:, :, :, 64:128], 1.0)

        # causal masks for the 4 diagonal offsets, generated on device:
        # dm4[p, di, par, j] = 1 if j - p - 128*di >= 0 else 0
        dm4 = consts.tile([128, 4, 2, QB], F16, name="dm4")
        nc.vector.memset(dm4[:], 1.0)
        for di in range(4):
            nc.gpsimd.affine_select(
                dm4[:, di],
                dm4[:, di],
                pattern=[[0, 2], [1, QB]],
                compare_op=mybir.AluOpType.is_ge,
                fill=0.0,
                base=-128 * di,
                channel_multiplier=-1,
            )

        # ---- prologue projections: emitted below (after the building
        # blocks are defined) so window 0's first score steps can be
        # interleaved between the k/q chains ----

        # prefetch: quarter-1 q/k gate window 1; xv0 is only needed by the
        # av burst at window 0's end, so it queues behind them
        xq_t[1] = x_quarter(xq_pool, xqT, 1, "xqq")
        xk_t[1] = x_quarter(xk_pool, xkT, 1, "xkq")
        wv0 = consts.tile([128, 4, HG], F16, name="wvt0")
        nc.sync.dma_start(wv0[:], wvT[:, 0:4, :])
        wv1 = consts.tile([128, 4, HG], F16, name="wvt1")
        nc.sync.dma_start(wv1[:], wvT[:, 4:8, :])
        w_sb["wv"] = [wv0, wv1]
        xv_t[0] = x_quarter(xv_pool, xvT, 0, "xvq")
        xv_t[1] = x_quarter(xv_pool, xvT, 1, "xvq")
        woT_sb = consts.tile([128, 2, D], F16, name="woTt")
        nc.sync.dma_start(woT_sb[:], woT[:])
        # prefetch two windows ahead: quarter Q+1's kq chains fill during
        # window Q, so its x must be fully resident by then
        xq_t[2] = x_quarter(xq_pool, xqT, 2, "xqq")
        xk_t[2] = x_quarter(xk_pool, xkT, 2, "xkq")
        xv_t[2] = x_quarter(xv_pool, xvT, 2, "xvq")

        # ---- building blocks (2-pass: one head-pair per pass) ----
        def sc_half(Q, kc, mi):
            """One head-pair's score matmuls + exp (+ diag mask) for one kc."""
            di = kc - 4 * Q
            sp = ps_sc.tile([128, 2, QB], F32, name="scp")
            for par in range(2):  # concurrent 64-row PE tiles
                h = 2 * mi + par
                lo = 64 * par
                nc.tensor.matmul(
                    sp[:, par, :],
                    kT_sb[lo : lo + 64, mi, ts(kc, 128)],
                    q_pad[h][lo : lo + 64, ts(Q, QB)],
                    start=True,
                    stop=True,
                )
            et = et_pool.tile([128, 2, QB], F16, name="et")
            nc.scalar.activation(
                et[:], sp[:], mybir.ActivationFunctionType.Exp, scale=0.125
            )
            if di >= 0:  # diagonal tile: multiplicative causal mask
                nc.vector.tensor_mul(et[:], et[:], dm4[:, di])
            return et

        def av2_step(av2, kc, et, mi, n_kc):
            for par in range(2):
                h = 2 * mi + par
                nc.tensor.matmul(
                    av2[:, par, :],
                    v_sb[:, kc, ts(h, 128)],
                    et[:, par, :],
                    start=(kc == 0),
                    stop=(kc == n_kc - 1),
                )

        def kq_tiles(Q):
            """Two aux-psum tiles (q then k) for quarter Q, each as 4 chunk
            thunks of 4 matmuls. Chunks of one tile must be emitted
            contiguously w.r.t. other aux allocations (single aux slot)."""
            tiles = []
            for wname, evac in (("wq", "q"), ("wk", "k")):
                state = {}

                def get_tile(state=state):
                    if "t" not in state:
                        state["t"] = ps_aux.tile([128, 2, QB], F32, name="aux")
                    return state["t"]

                chunks = []
                for mi in range(2):
                    for half in range(2):

                        def th(wname=wname, evac=evac, mi=mi, half=half, gt=get_tile):
                            t = gt()
                            xt = xq_t[Q] if wname == "wq" else xk_t[Q]
                            for kc in range(4 * half, 4 * half + 4):
                                nc.tensor.matmul(
                                    t[:, mi, :],
                                    ws(wname, kc)[:, ts(mi, 128)],
                                    xs(xt, kc),
                                    start=(kc == 0),
                                    stop=(kc == KC - 1),
                                )
                            if half == 1:
                                if evac == "k":
                                    nc.vector.tensor_scalar_add(
                                        kT_sb[:, mi, ts(Q, QB)],
                                        t[:, mi, :],
                                        bk_sb[:, mi : mi + 1],
                                    )
                                else:
                                    for par in range(2):
                                        h = 2 * mi + par
                                        lo = 64 * par
                                        nc.vector.tensor_scalar_add(
                                            q_pad[h][lo : lo + 64, ts(Q, QB)],
                                            t[lo : lo + 64, mi, :],
                                            bq_sb[lo : lo + 64, mi : mi + 1],
                                        )

                        chunks.append(th)
                tiles.append(chunks)
            return tiles

        def kq_half_tile(Q, wname, evac, mi):
            """Single-mi projection half as one aux tile of 2 chunk thunks."""
            state = {}

            def get_tile():
                if "t" not in state:
                    state["t"] = ps_aux.tile([128, QB], F32, name="aux")
                return state["t"]

            chunks = []
            for half in range(2):

                def th(half=half):
                    t = get_tile()
                    xt = xq_t[Q] if wname == "wq" else xk_t[Q]
                    for kc in range(4 * half, 4 * half + 4):
                        nc.tensor.matmul(
                            t[:],
                            ws(wname, kc)[:, ts(mi, 128)],
                            xs(xt, kc),
                            start=(kc == 0),
                            stop=(kc == KC - 1),
                        )
                    if half == 1:
                        if evac == "k":
                            nc.vector.tensor_scalar_add(
                                kT_sb[:, mi, ts(Q, QB)], t[:], bk_sb[:, mi : mi + 1]
                            )
                        else:
                            for par in range(2):
                                h = 2 * mi + par
                                lo = 64 * par
                                nc.vector.tensor_scalar_add(
                                    q_pad[h][lo : lo + 64, ts(Q, QB)],
                                    t[lo : lo + 64, :],
                                    bq_sb[lo : lo + 64, mi : mi + 1],
                                )

                chunks.append(th)
            return chunks

        def v_thunk(sc):
            def th():
                xt = xv_t[sc // 4]
                si = sc % 4
                ps = ps_aux.tile([128, 2, QB], F32, name="aux")[:, 0, 0:HG]
                for kc in range(KC):
                    nc.tensor.matmul(
                        ps[:],
                        xs(xt, kc)[:, ts(si, 128)],
                        ws("wv", kc),
                        start=(kc == 0),
                        stop=(kc == KC - 1),
                    )
                nc.vector.tensor_copy(
                    v_view[:, sc, :, 0:64], ps.rearrange("p (h x) -> p h x", x=64)[:]
                )

            return th

        def op_thunk(si):
            def th():
                ot = outsb.tile([128, D], F16, name="ot")
                otv = ot.rearrange("p (c n) -> p c n", c=2)
                pso = ps_aux.tile([128, 2, QB], F32, name="aux")
                for nj in range(2):
                    nc.tensor.matmul(
                        pso[:, nj, :],
                        attnT_sb[:, 0, ts(si, 128)],
                        woT_sb[:, 0, ts(nj, QB)],
                        start=True,
                        stop=False,
                    )
                nc.tensor.matmul(
                    pso[:, 0, :],
                    attnT_sb[:, 1, ts(si, 128)],
                    woT_sb[:, 1, ts(0, QB)],
                    start=False,
                    stop=True,
                )
                # nj0 evac overlaps nj1's final matmul: the aux slot frees
                # ~0.6us earlier, easing the projection serialization
                nc.vector.tensor_copy(otv[:, 0, :], pso[:, 0, :])
                nc.tensor.matmul(
                    pso[:, 1, :],
                    attnT_sb[:, 1, ts(si, 128)],
                    woT_sb[:, 1, ts(1, QB)],
                    start=False,
                    stop=True,
                )
                nc.vector.tensor_copy(otv[:, 1, :], pso[:, 1, :])
                nc.sync.dma_start(out_d[ts(si, 128), :], ot[:])

            return th

        def norm_release_pass(av2, Q, mi):
            # value rows out + Ln/Exp of the rowsums; overlaps the other
            # pass's attention on ACT
            c2 = small.tile([64, 2, QB], F16, name="csb")
            nc.vector.tensor_copy(c2[:], av2[0:64, :, :])
            lnrs = small.tile([64, 2, QB], F32, name="lnrs")
            nc.scalar.activation(
                lnrs[:], av2[64:128, :, :], mybir.ActivationFunctionType.Ln
            )
            rblk = small.tile([64, 2, QB], F16, name="rblk")
            nc.scalar.activation(
                rblk[:], lnrs[:], mybir.ActivationFunctionType.Exp, scale=-1.0
            )
            return c2, rblk

        def norm_muls_pass(Q, mi, c2, rblk):
            nc.vector.tensor_mul(
                attnT_sb[0:64, mi, ts(Q, QB)], c2[:, 0, :], rblk[:, 0, :]
            )
            stage_t = small.tile([64, QB], F16, name="stage_t")
            nc.vector.tensor_mul(stage_t[:], c2[:, 1, :], rblk[:, 1, :])
            nc.sync.dma_start(attnT_sb[64:128, mi, ts(Q, QB)], stage_t[:])

        # ---- global pipelined emission ----
        # Each window runs its query block twice (pass mi=0, then mi=1);
        # scores own the 2-slot ps_sc rotation exclusively (ACT-drained),
        # all projections serialize through the single aux slot, and the
        # av pair-accumulator lives in its own 2-bank slot.

        def window(Q, kq_ts, v_th, fillers, emit_norm_muls, pre_in):
            n_kc = 4 * Q + 4
            norms = []
            pre_out = {}
            # flatten kq tiles into (tile_idx, chunk) pairs for contiguity
            kq_flat = [(ti, th) for ti, tile in enumerate(kq_ts) for th in tile]
            aux_state = {"open": None}

            def pop_kq():
                ti, th = kq_flat.pop(0)
                th()
                nxt = kq_flat[0][0] if kq_flat else None
                aux_state["open"] = ti if nxt == ti else None

            def pop_aux_safe(queue):
                # finish an open kq tile before any other aux allocation
                while aux_state["open"] is not None and kq_flat:
                    pop_kq()
                if queue is v_th:
                    queue.pop(0)[1]()
                else:
                    queue.pop(0)()

            total_steps = 2 * n_kc
            kq_quota = len(kq_flat) / max(total_steps - 4, 1)
            f_quota = (len(v_th) + len(fillers)) / total_steps
            cr = {"kq": 0.0, "f": 0.0}

            for mi in (0, 1):
                ets_buf = dict(pre_in.get(mi, {}))
                av2 = ps_av.tile([128, 2, QB], F32, name="av2")
                state = {"next": 0}
                pre_b = {}

                def emit_pre(idx, mi=mi):
                    if mi == 0:
                        pre_b[idx] = sc_half(Q, idx, 1)
                    elif Q + 1 < NQB:
                        pre_out.setdefault(0, {})[idx] = sc_half(Q + 1, idx, 0)

                def drain_av(upto, mi=mi, av2=av2, ets_buf=ets_buf, state=state):
                    while state["next"] <= upto:
                        j = state["next"]
                        while v_th and v_th[0][0] <= j:
                            pop_aux_safe(v_th)
                        av2_step(av2, j, ets_buf.pop(j), mi, n_kc)
                        state["next"] += 1

                for kc in range(n_kc):
                    if kc not in ets_buf:
                        ets_buf[kc] = sc_half(Q, kc, mi)
                    if mi == 0 and kc == 0 and emit_norm_muls is not None:
                        emit_norm_muls()
                    if mi == 1 and kc == 0 and Q == NQB - 1:
                        # last window: pass A's normalize muls land during
                        # pass B so the tail out-proj ci0 can start at once
                        norm_muls_pass(Q, 0, *norms[0])
                    cr["kq"] += kq_quota
                    while cr["kq"] >= 1.0 and kq_flat:
                        pop_kq()
                        cr["kq"] -= 1.0
                    cr["f"] += f_quota
                    while cr["f"] >= 1.0 and (v_th or fillers):
                        if v_th and aux_state["open"] is None:
                            pop_aux_safe(v_th)
                        elif fillers and aux_state["open"] is None:
                            pop_aux_safe(fillers)
                        elif kq_flat:
                            pop_kq()
                        else:
                            break
                        cr["f"] -= 1.0
                    # final pass: tighten the av lag near the end so the
                    # rowsums (and thus the tail normalize) finish sooner
                    last = Q == NQB - 1 and mi == 1 and kc >= n_kc - 3
                    drain_av(kc - 1 if last else kc - 2)
                # pass end: pre-emit the next pass's (or next window's)
                # first two score steps so ACT stays fed through the
                # trailing avs + normalize
                # pass end: pre-emit the next pass's (or next window's)
                # first two score steps so ACT stays fed through the
                # trailing avs + normalize; a third follows the avs
                if mi == 1:
                    while kq_flat:  # q(Q+1) evacs must precede its scores
                        pop_kq()
                emit_pre(0)
                emit_pre(1)
                if Q == NQB - 1 and mi == 1:
                    # tail: trailing avs FIRST so the normalize's rowsum
                    # inputs aren't stuck behind leftover out-proj fillers
                    drain_av(n_kc - 1)
                if mi == 1:
                    while v_th:
                        pop_aux_safe(v_th)
                    while fillers:
                        pop_aux_safe(fillers)
                drain_av(n_kc - 1)
                emit_pre(2)
                norms.append(norm_release_pass(av2, Q, mi))
                if mi == 0:
                    pre_in = {1: pre_b}
            return norms, pre_out

        # prologue projections: k/q mi-halves as four sequential aux
        # tiles, with window 0's pass-A score steps emitted as soon as
        # the mi=0 halves land — the exp stream starts ~7us earlier
        # while the mi=1 chains and quarter-1 fills run underneath
        ets0 = {}
        for mi in range(2):
            kt = ps_aux.tile([128, QB], F32, name="aux")
            for kc in range(KC):
                nc.tensor.matmul(
                    kt[:],
                    ws("wk", kc)[:, ts(mi, 128)],
                    xs(xk_t[0], kc),
                    start=(kc == 0),
                    stop=(kc == KC - 1),
                )
            nc.vector.tensor_scalar_add(
                kT_sb[:, mi, ts(0, QB)], kt[:], bk_sb[:, mi : mi + 1]
            )
            qt = ps_aux.tile([128, QB], F32, name="aux")
            for kc in range(KC):
                nc.tensor.matmul(
                    qt[:],
                    ws("wq", kc)[:, ts(mi, 128)],
                    xs(xq_t[0], kc),
                    start=(kc == 0),
                    stop=(kc == KC - 1),
                )
            for par in range(2):
                h = 2 * mi + par
                lo = 64 * par
                nc.vector.tensor_scalar_add(
                    q_pad[h][lo : lo + 64, ts(0, QB)],
                    qt[lo : lo + 64, :],
                    bq_sb[lo : lo + 64, mi : mi + 1],
                )
            if mi == 0:
                ets0[0] = sc_half(0, 0, 0)
                ets0[1] = sc_half(0, 1, 0)
            else:
                ets0[2] = sc_half(0, 2, 0)
                ets0[3] = sc_half(0, 3, 0)

        # window 0: fillers are quarter-1 kq chains + v(0)
        norm, pre = window(
            0,
            kq_tiles(1),
            [(sc, v_thunk(sc)) for sc in range(4)],
            [],
            None,
            {0: ets0},
        )

        # out-projections split by per-window PE slack
        op_sched = {1: [0], 2: [1, 2], 3: list(range(3, 12))}
        for Q in range(1, NQB):
            # prefetch two windows ahead
            if Q + 2 < NQB:
                xq_t[Q + 2] = x_quarter(xq_pool, xqT, Q + 2, "xqq")
                xk_t[Q + 2] = x_quarter(xk_pool, xkT, Q + 2, "xkq")
                xv_t[Q + 2] = x_quarter(xv_pool, xvT, Q + 2, "xvq")
            kq_ts = kq_tiles(Q + 1) if Q + 1 < NQB else []
            v_th = [(sc, v_thunk(sc)) for sc in range(4 * Q, 4 * Q + 4)]
            fillers = [op_thunk(si) for si in op_sched[Q]]
            prev_norm = norm
            prev_Q = Q - 1

            def emit_muls(prev_norm=prev_norm, prev_Q=prev_Q):
                for mi in range(2):
                    norm_muls_pass(prev_Q, mi, *prev_norm[mi])

            norm, pre = window(Q, kq_ts, v_th, fillers, emit_muls, pre)

        # tail: last window's pass-B normalize + out-proj, ci-interleaved
        # (pass A's muls were emitted inside window 3's pass B)
        norm_muls_pass(NQB - 1, 1, *norm[1])
        for sg in ((12, 13), (14, 15)):
            psos = {}
            for si in sg:
                psos[si] = ps_sc.tile([128, 2, QB], F32, name="scp")
            for si in sg:  # ci0 chains start right after the mi0 muls
                for nj in range(2):
                    nc.tensor.matmul(
                        psos[si][:, nj, :],
                        attnT_sb[:, 0, ts(si, 128)],
                        woT_sb[:, 0, ts(nj, QB)],
                        start=True,
                        stop=False,
                    )
            for si in sg:  # ci1 chains with per-half evac overlap
                ot = outsb.tile([128, D], F16, name="ot")
                otv = ot.rearrange("p (c n) -> p c n", c=2)
                nc.tensor.matmul(
                    psos[si][:, 0, :],
                    attnT_sb[:, 1, ts(si, 128)],
                    woT_sb[:, 1, ts(0, QB)],
                    start=False,
                    stop=True,
                )
                nc.vector.tensor_copy(otv[:, 0, :], psos[si][:, 0, :])
                nc.tensor.matmul(
                    psos[si][:, 1, :],
                    attnT_sb[:, 1, ts(si, 128)],
                    woT_sb[:, 1, ts(1, QB)],
                    start=False,
                    stop=True,
                )
                nc.vector.tensor_copy(otv[:, 1, :], psos[si][:, 1, :])
                nc.sync.dma_start(out_d[ts(si, 128), :], ot[:])
    return nc


# --------------------------------------------------------------------------
# Host sharding / gathering
# --------------------------------------------------------------------------


def _pack_x(xT):
    # [1024, 2048] -> [128, quarter, kc, 512], contiguous per partition
    return np.ascontiguousarray(
        xT.reshape(KC, 128, NQB, QB).transpose(1, 2, 0, 3)
    ).astype(np.float16)


def _pack_w(wT):
    # [1024, 256] -> [128, kc, 256]
    return np.ascontiguousarray(wT.reshape(KC, 128, HG).transpose(1, 0, 2)).astype(
        np.float16
    )


def _make_in_maps(Q, K, V, Wq, bq, Wk, bk, Wv, bv, Wo):
    xT = {}
    for b in range(2):
        xT[b] = {
            "q": _pack_x(Q[b].T.astype(np.float32)),
            "k": _pack_x(K[b].T.astype(np.float32)),
            "v": _pack_x(V[b].T.astype(np.float32)),
        }
    in_maps = []
    for c in range(8):
        b, g = divmod(c, 4)
        sl = slice(HG * g, HG * (g + 1))
        in_maps.append(
            {
                "xqT": xT[b]["q"],
                "xkT": xT[b]["k"],
                "xvT": xT[b]["v"],
                "wqT": _pack_w(Wq[sl, :].T),
                "wkT": _pack_w(Wk[sl, :].T),
                "wvT": _pack_w(Wv[sl, :].T),
                "woT": np.ascontiguousarray(
                    Wo[:, sl].T.reshape(2, 128, D).transpose(1, 0, 2)
                ).astype(np.float16),
                "bq": np.ascontiguousarray(bq[sl].reshape(2, 128).T).astype(np.float32),
                "bk": np.ascontiguousarray(bk[sl].reshape(2, 128).T).astype(np.float32),
            }
        )
    return in_maps


_nc_cache = None


def kernel(Q, K, V, mask, Wq, bq, Wk, bk, Wv, bv, Wo, bo, **_unused):
    """Full inputs in, full [2, 2048, 1024] float32 output out.

    `mask` is the causal tril mask from setup_inputs(); causality is baked
    into the kernel structure (lower-triangular tiles only + diagonal-tile
    masking), so the tensor itself is not shipped to the device.
    """
    global _nc_cache
    _apply_patches()

    Q, K, V = (np.asarray(x, np.float32) for x in (Q, K, V))
    Wq, Wk, Wv, Wo = (np.asarray(x, np.float32) for x in (Wq, Wk, Wv, Wo))
    bq, bk, bv, bo = (np.asarray(x, np.float32) for x in (bq, bk, bv, bo))

    if _nc_cache is None:
        _nc_cache = _build()
    in_maps = _make_in_maps(Q, K, V, Wq, bq, Wk, bk, Wv, bv, Wo)
    res = bass_utils.run_bass_kernel_spmd(
        _nc_cache, in_maps, core_ids=list(range(8)), trace=False
    )
    out = np.zeros((2, S, D), np.float32)
    for c in range(8):
        out[c // 4] += res.results[c]["out"].astype(np.float32)
    # v-bias folded out of the device program: attn rows sum to 1, so
    # attn_true @ Wo^T = attn_nobias @ Wo^T + bv @ Wo^T
    out += (bo + bv @ Wo.T)[None, None, :]
    return out


# revision 49
# speedup vs baseline: 1.0149x; 1.0149x over previous
"""Sharded causal multi-head attention for 8 Trainium2 NeuronCores.

kernel(**inputs) takes the FULL inputs (Q, K, V, mask, Wq, bq, Wk, bk,
Wv, bv, Wo, bo) and returns the FULL [2, 2048, 1024] float32 output.

Sharding (data + head/tensor parallel): core c = 4*b + g handles batch
b in {0,1} and head-group g in {0..3} (4 heads, 256 dims). W_q/W_k/W_v
are column-parallel, W_o row-parallel; the host sums the 4 per-batch
row-parallel partials and adds (bo + bv @ Wo.T) - the v-bias commutes
out of the softmax-weighted sum because prob rows sum to 1.

v5 structure (2-pass global software pipeline):
  - ScalarE exp is the irreducible pacer (~89us of exp tiles); all other
    work is scheduled around keeping it fed.
  - Each query-block window runs TWICE (one head-pair per pass), so the
    av accumulator needs only 2 PSUM banks. That frees a dedicated
    2-bank "aux" slot for every projection (k/q chains, v, out-proj),
    leaving the 2-slot score rotation exclusively ACT-drained - the PE
    score stream is never stalled behind a DVE projection evacuation.
  - k/q projections for quarter Q+1 stream through window Q as
    fine-grained filler thunks; out-projections are deferred to the
    ACT-bound later windows; v fills in its own window.
  - x DMAs are throttled by pool-slot reuse (xq/xk bufs=6, xv bufs=4)
    so prefetch never steals HBM bandwidth from the critical quarter.
  - causal masks are generated on device (GpSimd affine_select into
    dm4 at startup) instead of a 1MB host DMA.
  - pass/window boundaries pre-emit the next pass's first score steps
    so ACT runs through the trailing avs + normalize; a 10-matmul
    warmup unthrottles the PE clock (HAM) before the first projections.
"""

import json
import sys

for _p in ("/opt/trn_rl_repo", "/opt/trn_rl_repo/concourse"):
    if _p not in sys.path:
        sys.path.insert(0, _p)

import numpy as np

import bass_rust
import concourse.bass as bass
import concourse.mybir as mybir
import concourse.tile as tile
from concourse import bass_utils
from concourse.bass import ts
from concourse.vector_clock import ScopedClock

F32 = mybir.dt.float32
F16 = mybir.dt.float16  # 10-bit mantissa; every intermediate is O(1)-bounded
S = 2048
D = 1024
HG = 256  # head-group dims (4 heads x 64)
NH = 4  # heads per core
KC = D // 128
NQB = 4
QB = 512
NSC = S // 128

# --------------------------------------------------------------------------
# Environment patches: this container's walrus accepts only ONE sync-wait
# command per instruction, but Tile emits several (and its epilogue drain
# carries one per outstanding proc sem). Split extras onto single-wait NoOps.
# --------------------------------------------------------------------------

_patched = False


def _drain_and_barrier_split(self, tick_clock, wait_clock):
    nc = self.nc
    probe = nc.sync.nop()
    wait_clock.add_sem_waits(probe.ins, ScopedClock({None: tick_clock.global_clock}))
    si = probe.ins.sync_info
    waits = list(si.on_wait) if si is not None and si.on_wait else []
    if len(waits) > 1:
        si.on_wait = [waits[0]]
        for w in waits[1:]:
            nop = nc.sync.nop()
            nop.ins.sync_info = bass_rust.SyncInfo(on_wait=[w], on_update=[])
    nc.sync.drain()
    nc.all_engine_barrier()
    assert self.sems is not None
    popped = nc._tile_sem_poison_stack.pop()
    assert popped is self._sem_poison
    nc.clear_and_free_semaphores(list(self.sems.allocated().values()))
    nc.all_engine_barrier()


def _split_waits_json(raw):
    j = json.loads(raw)
    changed = False
    for f in j.get("functions", []):
        for bb in f.get("blocks", []):
            out = []
            for inst in bb.get("instructions", []):
                si = inst.get("sync_info")
                waits = (si or {}).get("on_wait") or []
                if len(waits) > 1:
                    for k, w in enumerate(waits[:-1]):
                        nop = {
                            "engine": inst["engine"],
                            "ins": [],
                            "name": f"{inst['name']}-ws{k}",
                            "opcode": "NoOp",
                            "outs": [],
                            "sync_info": {"on_update": [], "on_wait": [w]},
                        }
                        if "debug" in inst:
                            nop["debug"] = inst["debug"]
                        out.append(nop)
                    si["on_wait"] = [waits[-1]]
                    changed = True
                out.append(inst)
            if changed:
                bb["instructions"] = out
    return json.dumps(j).encode() if changed else raw


def _apply_patches():
    global _patched
    if _patched:
        return
    tile.TileContext._drain_and_barrier = _drain_and_barrier_split
    orig_to_json = bass.Bass.to_json_bytes
    bass.Bass.to_json_bytes = lambda self: _split_waits_json(orig_to_json(self))
    # NOTE: do NOT enable walrus ldw-opt here - it crashes codegen
    # (visitInstLdweights) for 2-byte matmul dtypes.
    _patched = True


# --------------------------------------------------------------------------
# Per-core Bass program
# --------------------------------------------------------------------------


def _build():
    nc = bass.Bass("TRN2", target_bir_lowering=False, debug=False, num_devices=8)

    # all host-packed: partition dim first, fully contiguous per partition
    xqT = nc.dram_tensor("xqT", [128, NQB, KC, QB], F16, kind="ExternalInput").ap()
    xkT = nc.dram_tensor("xkT", [128, NQB, KC, QB], F16, kind="ExternalInput").ap()
    xvT = nc.dram_tensor("xvT", [128, NQB, KC, QB], F16, kind="ExternalInput").ap()
    wqT = nc.dram_tensor("wqT", [128, KC, HG], F16, kind="ExternalInput").ap()
    wkT = nc.dram_tensor("wkT", [128, KC, HG], F16, kind="ExternalInput").ap()
    wvT = nc.dram_tensor("wvT", [128, KC, HG], F16, kind="ExternalInput").ap()
    woT = nc.dram_tensor("woT", [128, 2, D], F16, kind="ExternalInput").ap()
    bq_d = nc.dram_tensor("bq", [128, 2], F32, kind="ExternalInput").ap()
    bk_d = nc.dram_tensor("bk", [128, 2], F32, kind="ExternalInput").ap()
    out_d = nc.dram_tensor("out", [S, D], F16, kind="ExternalOutput").ap()

    from contextlib import ExitStack

    with tile.TileContext(nc) as tc, ExitStack() as ctx:
        consts = ctx.enter_context(tc.tile_pool(name="consts", bufs=1))
        qkv_sb = ctx.enter_context(tc.tile_pool(name="qkv", bufs=1))
        # xq/xk at 6 bufs (1.5 quarters): a later quarter's chunk DMA waits
        # (via slot reuse) until the matching earlier chunk has been read,
        # so prefetch never steals HBM bandwidth from the critical quarter
        xk_pool = ctx.enter_context(tc.tile_pool(name="xk", bufs=6))
        xq_pool = ctx.enter_context(tc.tile_pool(name="xq", bufs=6))
        xv_pool = ctx.enter_context(tc.tile_pool(name="xv", bufs=4))
        et_pool = ctx.enter_context(tc.tile_pool(name="et", bufs=16))
        small = ctx.enter_context(tc.tile_pool(name="small", bufs=2))
        outsb = ctx.enter_context(tc.tile_pool(name="outsb", bufs=3))

        # PSUM (8 banks): 2 score slots (2 banks each, ACT-drained only) +
        # one aux slot (2 banks: kq/v/out projections, DVE-drained) + one
        # av pair-accumulator slot (2 banks)
        ps_sc = ctx.enter_context(tc.tile_pool(name="ps_sc", bufs=2, space="PSUM"))
        ps_aux = ctx.enter_context(tc.tile_pool(name="ps_aux", bufs=1, space="PSUM"))
        ps_av = ctx.enter_context(tc.tile_pool(name="ps_av", bufs=1, space="PSUM"))

        # ---- DMA helpers: chunks are >=2KB contiguous per partition ----
        def x_quarter(pool, dram, quarter, name):
            chunks = []
            for c4 in range(4):
                t = pool.tile([128, 2, QB], F16, name=name)
                nc.sync.dma_start(t[:], dram[:, quarter, 2 * c4 : 2 * c4 + 2, :])
                chunks.append(t)
            return chunks

        def xs(chunks, kc):
            return chunks[kc // 2][:, kc % 2, :]

        def w_load_interleaved(name, dram, xpool, xdram, xname):
            # wc0, xc0, xc1, wc1, xc2, xc3: first matmul needs only wc0+xc0
            wchunks, xchunks = [], []
            wt = consts.tile([128, 4, HG], F16, name=f"{name}0")
            nc.sync.dma_start(wt[:], dram[:, 0:4, :])
            wchunks.append(wt)
            for c4 in range(4):
                t = xpool.tile([128, 2, QB], F16, name=xname)
                nc.sync.dma_start(t[:], xdram[:, 0, 2 * c4 : 2 * c4 + 2, :])
                xchunks.append(t)
                if c4 == 1:
                    wt = consts.tile([128, 4, HG], F16, name=f"{name}1")
                    nc.sync.dma_start(wt[:], dram[:, 4:8, :])
                    wchunks.append(wt)
            return wchunks, xchunks

        # ---- prologue DMAs: quarter-0 k/q first (they gate everything) ----
        w_sb = {}
        xk_t, xq_t, xv_t = {}, {}, {}
        w_sb["wk"], xk_t[0] = w_load_interleaved("wkt", wkT, xk_pool, xkT, "xkq")
        bk_sb = consts.tile([128, 2], F32, name="bkt")
        nc.sync.dma_start(bk_sb[:], bk_d[:])
        w_sb["wq"], xq_t[0] = w_load_interleaved("wqt", wqT, xq_pool, xqT, "xqq")
        bq_sb = consts.tile([128, 2], F32, name="bqt")
        nc.sync.dma_start(bq_sb[:], bq_d[:])

        def ws(name, kc):
            return w_sb[name][kc // 4][:, kc % 4, :]

        # ACT table warmup: load the natural_log_exp set before it matters
        warm = consts.tile([128, 8], F32, name="warm")
        nc.vector.memset(warm[:], 1.0)
        warm2 = consts.tile([128, 8], F16, name="warm2")
        nc.scalar.activation(warm2[:], warm[:], mybir.ActivationFunctionType.Exp)

        # PE clock warmup: the HAM clock-gate holds the PE at 1.2GHz until
        # it has seen a ~3.4us busy window. The prologue projections land
        # right when the first DMAs complete; without this they all run at
        # half clock. ~10 dummy matmuls (~5us cold) warm the array while
        # the first x/w chunks are still in flight.
        wsc = consts.tile([128, QB], F16, name="wsc")
        nc.vector.memset(wsc[:], 0.125)
        warm_ps = ps_aux.tile([128, 2, QB], F32, name="aux")
        for r in range(10):
            nc.tensor.matmul(
                warm_ps[:, 0, :],
                wsc[:, 0:128],
                wsc[:],
                start=(r == 0),
                stop=(r == 9),
            )
        nc.vector.tensor_copy(wsc[:], warm_ps[:, 0, :])

        # ---- persistent activations ----
        q_pad = [qkv_sb.tile([128, S], F16, name=f"qp{h}") for h in range(NH)]
        kT_sb = qkv_sb.tile([128, 2, S], F16, name="kT")
        v_sb = qkv_sb.tile([128, NSC, NH * 128], F16, name="vp")
        attnT_sb = qkv_sb.tile([128, 2, S], F16, name="attnT")

        v_view = v_sb.rearrange("p c (h x) -> p c h x", x=128)
        nc.vector.memset(v_view[:, :, :, 64:128], 1.0)

        # causal masks for the 4 diagonal offsets, generated on device:
        # dm4[p, di, par, j] = 1 if j - p - 128*di >= 0 else 0
        dm4 = consts.tile([128, 4, 2, QB], F16, name="dm4")
        nc.vector.memset(dm4[:], 1.0)
        for di in range(4):
            nc.gpsimd.affine_select(
                dm4[:, di],
                dm4[:, di],
                pattern=[[0, 2], [1, QB]],
                compare_op=mybir.AluOpType.is_ge,
                fill=0.0,
                base=-128 * di,
                channel_multiplier=-1,
            )

        # ---- prologue projections: emitted below (after the building
        # blocks are defined) so window 0's first score steps can be
        # interleaved between the k/q chains ----

        # prefetch: quarter-1 q/k gate window 1; xv0 is only needed by the
        # av burst at window 0's end, so it queues behind them
        xq_t[1] = x_quarter(xq_pool, xqT, 1, "xqq")
        xk_t[1] = x_quarter(xk_pool, xkT, 1, "xkq")
        wv0 = consts.tile([128, 4, HG], F16, name="wvt0")
        nc.sync.dma_start(wv0[:], wvT[:, 0:4, :])
        wv1 = consts.tile([128, 4, HG], F16, name="wvt1")
        nc.sync.dma_start(wv1[:], wvT[:, 4:8, :])
        w_sb["wv"] = [wv0, wv1]
        xv_t[0] = x_quarter(xv_pool, xvT, 0, "xvq")
        xv_t[1] = x_quarter(xv_pool, xvT, 1, "xvq")
        woT_sb = consts.tile([128, 2, D], F16, name="woTt")
        nc.sync.dma_start(woT_sb[:], woT[:])
        # prefetch two windows ahead: quarter Q+1's kq chains fill during
        # window Q, so its x must be fully resident by then
        xq_t[2] = x_quarter(xq_pool, xqT, 2, "xqq")
        xk_t[2] = x_quarter(xk_pool, xkT, 2, "xkq")
        xv_t[2] = x_quarter(xv_pool, xvT, 2, "xvq")

        # ---- building blocks (2-pass: one head-pair per pass) ----
        def sc_half(Q, kc, mi):
            """One head-pair's score matmuls + exp (+ diag mask) for one kc."""
            di = kc - 4 * Q
            sp = ps_sc.tile([128, 2, QB], F32, name="scp")
            for par in range(2):  # concurrent 64-row PE tiles
                h = 2 * mi + par
                lo = 64 * par
                nc.tensor.matmul(
                    sp[:, par, :],
                    kT_sb[lo : lo + 64, mi, ts(kc, 128)],
                    q_pad[h][lo : lo + 64, ts(Q, QB)],
                    start=True,
                    stop=True,
                )
            et = et_pool.tile([128, 2, QB], F16, name="et")
            nc.scalar.activation(
                et[:], sp[:], mybir.ActivationFunctionType.Exp, scale=0.125
            )
            if di >= 0:  # diagonal tile: multiplicative causal mask
                nc.vector.tensor_mul(et[:], et[:], dm4[:, di])
            return et

        def av2_step(av2, kc, et, mi, n_kc):
            for par in range(2):
                h = 2 * mi + par
                nc.tensor.matmul(
                    av2[:, par, :],
                    v_sb[:, kc, ts(h, 128)],
                    et[:, par, :],
                    start=(kc == 0),
                    stop=(kc == n_kc - 1),
                )

        def kq_tiles(Q):
            """Two aux-psum tiles (q then k) for quarter Q, each as 4 chunk
            thunks of 4 matmuls. Chunks of one tile must be emitted
            contiguously w.r.t. other aux allocations (single aux slot)."""
            tiles = []
            for wname, evac in (("wq", "q"), ("wk", "k")):
                state = {}

                def get_tile(state=state):
                    if "t" not in state:
                        state["t"] = ps_aux.tile([128, 2, QB], F32, name="aux")
                    return state["t"]

                chunks = []
                for mi in range(2):
                    for half in range(2):

                        def th(wname=wname, evac=evac, mi=mi, half=half, gt=get_tile):
                            t = gt()
                            xt = xq_t[Q] if wname == "wq" else xk_t[Q]
                            for kc in range(4 * half, 4 * half + 4):
                                nc.tensor.matmul(
                                    t[:, mi, :],
                                    ws(wname, kc)[:, ts(mi, 128)],
                                    xs(xt, kc),
                                    start=(kc == 0),
                                    stop=(kc == KC - 1),
                                )
                            if half == 1:
                                if evac == "k":
                                    nc.vector.tensor_scalar_add(
                                        kT_sb[:, mi, ts(Q, QB)],
                                        t[:, mi, :],
                                        bk_sb[:, mi : mi + 1],
                                    )
                                else:
                                    for par in range(2):
                                        h = 2 * mi + par
                                        lo = 64 * par
                                        nc.vector.tensor_scalar_add(
                                            q_pad[h][lo : lo + 64, ts(Q, QB)],
                                            t[lo : lo + 64, mi, :],
                                            bq_sb[lo : lo + 64, mi : mi + 1],
                                        )

                        chunks.append(th)
                tiles.append(chunks)
            return tiles

        def kq_half_tile(Q, wname, evac, mi):
            """Single-mi projection half as one aux tile of 2 chunk thunks."""
            state = {}

            def get_tile():
                if "t" not in state:
                    state["t"] = ps_aux.tile([128, QB], F32, name="aux")
                return state["t"]

            chunks = []
            for half in range(2):

                def th(half=half):
                    t = get_tile()
                    xt = xq_t[Q] if wname == "wq" else xk_t[Q]
                    for kc in range(4 * half, 4 * half + 4):
                        nc.tensor.matmul(
                            t[:],
                            ws(wname, kc)[:, ts(mi, 128)],
                            xs(xt, kc),
                            start=(kc == 0),
                            stop=(kc == KC - 1),
                        )
                    if half == 1:
                        if evac == "k":
                            nc.vector.tensor_scalar_add(
                                kT_sb[:, mi, ts(Q, QB)], t[:], bk_sb[:, mi : mi + 1]
                            )
                        else:
                            for par in range(2):
                                h = 2 * mi + par
                                lo = 64 * par
                                nc.vector.tensor_scalar_add(
                                    q_pad[h][lo : lo + 64, ts(Q, QB)],
                                    t[lo : lo + 64, :],
                                    bq_sb[lo : lo + 64, mi : mi + 1],
                                )

                chunks.append(th)
            return chunks

        def v_thunk(sc):
            def th():
                xt = xv_t[sc // 4]
                si = sc % 4
                ps = ps_aux.tile([128, 2, QB], F32, name="aux")[:, 0, 0:HG]
                for kc in range(KC):
                    nc.tensor.matmul(
                        ps[:],
                        xs(xt, kc)[:, ts(si, 128)],
                        ws("wv", kc),
                        start=(kc == 0),
                        stop=(kc == KC - 1),
                    )
                nc.vector.tensor_copy(
                    v_view[:, sc, :, 0:64], ps.rearrange("p (h x) -> p h x", x=64)[:]
                )

            return th

        def op_thunk(si):
            def th():
                ot = outsb.tile([128, D], F16, name="ot")
                otv = ot.rearrange("p (c n) -> p c n", c=2)
                pso = ps_aux.tile([128, 2, QB], F32, name="aux")
                for nj in range(2):
                    nc.tensor.matmul(
                        pso[:, nj, :],
                        attnT_sb[:, 0, ts(si, 128)],
                        woT_sb[:, 0, ts(nj, QB)],
                        start=True,
                        stop=False,
                    )
                nc.tensor.matmul(
                    pso[:, 0, :],
                    attnT_sb[:, 1, ts(si, 128)],
                    woT_sb[:, 1, ts(0, QB)],
                    start=False,
                    stop=True,
                )
                # nj0 evac overlaps nj1's final matmul: the aux slot frees
                # ~0.6us earlier, easing the projection serialization
                nc.vector.tensor_copy(otv[:, 0, :], pso[:, 0, :])
                nc.tensor.matmul(
                    pso[:, 1, :],
                    attnT_sb[:, 1, ts(si, 128)],
                    woT_sb[:, 1, ts(1, QB)],
                    start=False,
                    stop=True,
                )
                nc.vector.tensor_copy(otv[:, 1, :], pso[:, 1, :])
                nc.sync.dma_start(out_d[ts(si, 128), :], ot[:])

            return th

        def norm_release_pass(av2, Q, mi):
            # value rows out + Ln/Exp of the rowsums; overlaps the other
            # pass's attention on ACT
            c2 = small.tile([64, 2, QB], F16, name="csb")
            nc.vector.tensor_copy(c2[:], av2[0:64, :, :])
            lnrs = small.tile([64, 2, QB], F32, name="lnrs")
            nc.scalar.activation(
                lnrs[:], av2[64:128, :, :], mybir.ActivationFunctionType.Ln
            )
            rblk = small.tile([64, 2, QB], F16, name="rblk")
            nc.scalar.activation(
                rblk[:], lnrs[:], mybir.ActivationFunctionType.Exp, scale=-1.0
            )
            return c2, rblk

        def norm_muls_pass(Q, mi, c2, rblk):
            nc.vector.tensor_mul(
                attnT_sb[0:64, mi, ts(Q, QB)], c2[:, 0, :], rblk[:, 0, :]
            )
            stage_t = small.tile([64, QB], F16, name="stage_t")
            nc.vector.tensor_mul(stage_t[:], c2[:, 1, :], rblk[:, 1, :])
            nc.sync.dma_start(attnT_sb[64:128, mi, ts(Q, QB)], stage_t[:])

        # ---- global pipelined emission ----
        # Each window runs its query block twice (pass mi=0, then mi=1);
        # scores own the 2-slot ps_sc rotation exclusively (ACT-drained),
        # all projections serialize through the single aux slot, and the
        # av pair-accumulator lives in its own 2-bank slot.

        def window(Q, kq_ts, v_th, fillers, emit_norm_muls, pre_in):
            n_kc = 4 * Q + 4
            norms = []
            pre_out = {}
            # flatten kq tiles into (tile_idx, chunk) pairs for contiguity
            kq_flat = [(ti, th) for ti, tile in enumerate(kq_ts) for th in tile]
            aux_state = {"open": None}

            def pop_kq():
                ti, th = kq_flat.pop(0)
                th()
                nxt = kq_flat[0][0] if kq_flat else None
                aux_state["open"] = ti if nxt == ti else None

            def pop_aux_safe(queue):
                # finish an open kq tile before any other aux allocation
                while aux_state["open"] is not None and kq_flat:
                    pop_kq()
                if queue is v_th:
                    queue.pop(0)[1]()
                else:
                    queue.pop(0)()

            total_steps = 2 * n_kc
            kq_quota = len(kq_flat) / max(total_steps - 4, 1)
            f_quota = (len(v_th) + len(fillers)) / total_steps
            cr = {"kq": 0.0, "f": 0.0}

            for mi in (0, 1):
                ets_buf = dict(pre_in.get(mi, {}))
                av2 = ps_av.tile([128, 2, QB], F32, name="av2")
                state = {"next": 0}
                pre_b = {}

                def emit_pre(idx, mi=mi):
                    if mi == 0:
                        pre_b[idx] = sc_half(Q, idx, 1)
                    elif Q + 1 < NQB:
                        pre_out.setdefault(0, {})[idx] = sc_half(Q + 1, idx, 0)

                def drain_av(upto, mi=mi, av2=av2, ets_buf=ets_buf, state=state):
                    while state["next"] <= upto:
                        j = state["next"]
                        while v_th and v_th[0][0] <= j:
                            pop_aux_safe(v_th)
                        av2_step(av2, j, ets_buf.pop(j), mi, n_kc)
                        state["next"] += 1

                for kc in range(n_kc):
                    if kc not in ets_buf:
                        ets_buf[kc] = sc_half(Q, kc, mi)
                    if mi == 0 and kc == 0 and emit_norm_muls is not None:
                        emit_norm_muls()
                    if mi == 1 and kc == 0 and Q == NQB - 1:
                        # last window: pass A's normalize muls land during
                        # pass B so the tail out-proj ci0 can start at once
                        norm_muls_pass(Q, 0, *norms[0])
                    cr["kq"] += kq_quota
                    while cr["kq"] >= 1.0 and kq_flat:
                        pop_kq()
                        cr["kq"] -= 1.0
                    cr["f"] += f_quota
                    while cr["f"] >= 1.0 and (v_th or fillers):
                        if v_th and aux_state["open"] is None:
                            pop_aux_safe(v_th)
                        elif fillers and aux_state["open"] is None:
                            pop_aux_safe(fillers)
                        elif kq_flat:
                            pop_kq()
                        else:
                            break
                        cr["f"] -= 1.0
                    # final pass: tighten the av lag near the end so the
                    # rowsums (and thus the tail normalize) finish sooner
                    last = Q == NQB - 1 and mi == 1 and kc >= n_kc - 3
                    drain_av(kc - 1 if last else kc - 2)
                # pass end: pre-emit the next pass's (or next window's)
                # first two score steps so ACT stays fed through the
                # trailing avs + normalize
                # pass end: pre-emit the next pass's (or next window's)
                # first two score steps so ACT stays fed through the
                # trailing avs + normalize; a third follows the avs
                if mi == 1:
                    while kq_flat:  # q(Q+1) evacs must precede its scores
                        pop_kq()
                emit_pre(0)
                emit_pre(1)
                if Q == NQB - 1 and mi == 1:
                    # tail: trailing avs FIRST so the normalize's rowsum
                    # inputs aren't stuck behind leftover out-proj fillers
                    drain_av(n_kc - 1)
                if mi == 1:
                    while v_th:
                        pop_aux_safe(v_th)
                    while fillers:
                        pop_aux_safe(fillers)
                drain_av(n_kc - 1)
                emit_pre(2)
                norms.append(norm_release_pass(av2, Q, mi))
                if mi == 0:
                    pre_in = {1: pre_b}
            return norms, pre_out

        # prologue projections: k/q mi-halves as four sequential aux
        # tiles, with window 0's pass-A score steps emitted as soon as
        # the mi=0 halves land — the exp stream starts ~7us earlier
        # while the mi=1 chains and quarter-1 fills run underneath
        ets0 = {}
        for mi in range(2):
            kt = ps_aux.tile([128, QB], F32, name="aux")
            for kc in range(KC):
                nc.tensor.matmul(
                    kt[:],
                    ws("wk", kc)[:, ts(mi, 128)],
                    xs(xk_t[0], kc),
                    start=(kc == 0),
                    stop=(kc == KC - 1),
                )
            nc.vector.tensor_scalar_add(
                kT_sb[:, mi, ts(0, QB)], kt[:], bk_sb[:, mi : mi + 1]
            )
            qt = ps_aux.tile([128, QB], F32, name="aux")
            for kc in range(KC):
                nc.tensor.matmul(
                    qt[:],
                    ws("wq", kc)[:, ts(mi, 128)],
                    xs(xq_t[0], kc),
                    start=(kc == 0),
                    stop=(kc == KC - 1),
                )
            for par in range(2):
                h = 2 * mi + par
                lo = 64 * par
                nc.vector.tensor_scalar_add(
                    q_pad[h][lo : lo + 64, ts(0, QB)],
                    qt[lo : lo + 64, :],
                    bq_sb[lo : lo + 64, mi : mi + 1],
                )
            if mi == 0:
                ets0[0] = sc_half(0, 0, 0)
                ets0[1] = sc_half(0, 1, 0)
            else:
                ets0[2] = sc_half(0, 2, 0)
                ets0[3] = sc_half(0, 3, 0)

        # window 0: fillers are quarter-1 kq chains + v(0)
        norm, pre = window(
            0,
            kq_tiles(1),
            [(sc, v_thunk(sc)) for sc in range(4)],
            [],
            None,
            {0: ets0},
        )

        # out-projections split by per-window PE slack
        op_sched = {1: [0], 2: [1, 2], 3: list(range(3, 12))}
        for Q in range(1, NQB):
            # prefetch two windows ahead
            if Q + 2 < NQB:
                xq_t[Q + 2] = x_quarter(xq_pool, xqT, Q + 2, "xqq")
                xk_t[Q + 2] = x_quarter(xk_pool, xkT, Q + 2, "xkq")
                xv_t[Q + 2] = x_quarter(xv_pool, xvT, Q + 2, "xvq")
            kq_ts = kq_tiles(Q + 1) if Q + 1 < NQB else []
            v_th = [(sc, v_thunk(sc)) for sc in range(4 * Q, 4 * Q + 4)]
            fillers = [op_thunk(si) for si in op_sched[Q]]
            prev_norm = norm
            prev_Q = Q - 1

            def emit_muls(prev_norm=prev_norm, prev_Q=prev_Q):
                for mi in range(2):
                    norm_muls_pass(prev_Q, mi, *prev_norm[mi])

            norm, pre = window(Q, kq_ts, v_th, fillers, emit_muls, pre)

        # tail: last window's pass-B normalize + out-proj, ci-interleaved
        # (pass A's muls were emitted inside window 3's pass B)
        norm_muls_pass(NQB - 1, 1, *norm[1])
        for sg in ((12, 13), (14, 15)):
            psos = {}
            for si in sg:
                psos[si] = ps_sc.tile([128, 2, QB], F32, name="scp")
            for si in sg:  # ci0 chains start right after the mi0 muls
                for nj in range(2):
                    nc.tensor.matmul(
                        psos[si][:, nj, :],
                        attnT_sb[:, 0, ts(si, 128)],
                        woT_sb[:, 0, ts(nj, QB)],
                        start=True,
                        stop=False,
                    )
            for si in sg:  # ci1 chains with per-half evac overlap
                ot = outsb.tile([128, D], F16, name="ot")
                otv = ot.rearrange("p (c n) -> p c n", c=2)
                nc.tensor.matmul(
                    psos[si][:, 0, :],
                    attnT_sb[:, 1, ts(si, 128)],
                    woT_sb[:, 1, ts(0, QB)],
                    start=False,
                    stop=True,
                )
                nc.vector.tensor_copy(otv[:, 0, :], psos[si][:, 0, :])
                nc.tensor.matmul(
                    psos[si][:, 1, :],
                    attnT_sb[:, 1, ts(si, 128)],
                    woT_sb[:, 1, ts(1, QB)],
                    start=False,
                    stop=True,
                )
                nc.vector.tensor_copy(otv[:, 1, :], psos[si][:, 1, :])
                nc.sync.dma_start(out_d[ts(si, 128), :], ot[:])
    return nc


# --------------------------------------------------------------------------
# Host sharding / gathering
# --------------------------------------------------------------------------


def _pack_x(xT):
    # [1024, 2048] -> [128, quarter, kc, 512], contiguous per partition
    return np.ascontiguousarray(
        xT.reshape(KC, 128, NQB, QB).transpose(1, 2, 0, 3)
    ).astype(np.float16)


def _pack_w(wT):
    # [1024, 256] -> [128, kc, 256]
    return np.ascontiguousarray(wT.reshape(KC, 128, HG).transpose(1, 0, 2)).astype(
        np.float16
    )


def _make_in_maps(Q, K, V, Wq, bq, Wk, bk, Wv, bv, Wo):
    xT = {}
    for b in range(2):
        xT[b] = {
            "q": _pack_x(Q[b].T.astype(np.float32)),
            "k": _pack_x(K[b].T.astype(np.float32)),
            "v": _pack_x(V[b].T.astype(np.float32)),
        }
    in_maps = []
    for c in range(8):
        b, g = divmod(c, 4)
        sl = slice(HG * g, HG * (g + 1))
        in_maps.append(
            {
                "xqT": xT[b]["q"],
                "xkT": xT[b]["k"],
                "xvT": xT[b]["v"],
                "wqT": _pack_w(Wq[sl, :].T),
                "wkT": _pack_w(Wk[sl, :].T),
                "wvT": _pack_w(Wv[sl, :].T),
                "woT": np.ascontiguousarray(
                    Wo[:, sl].T.reshape(2, 128, D).transpose(1, 0, 2)
                ).astype(np.float16),
                "bq": np.ascontiguousarray(bq[sl].reshape(2, 128).T).astype(np.float32),
                "bk": np.ascontiguousarray(bk[sl].reshape(2, 128).T).astype(np.float32),
            }
        )
    return in_maps


_nc_cache = None


def kernel(Q, K, V, mask, Wq, bq, Wk, bk, Wv, bv, Wo, bo, **_unused):
    """Full inputs in, full [2, 2048, 1024] float32 output out.

    `mask` is the causal tril mask from setup_inputs(); causality is baked
    into the kernel structure (lower-triangular tiles only + diagonal-tile
    masking), so the tensor itself is not shipped to the device.
    """
    global _nc_cache
    _apply_patches()

    Q, K, V = (np.asarray(x, np.float32) for x in (Q, K, V))
    Wq, Wk, Wv, Wo = (np.asarray(x, np.float32) for x in (Wq, Wk, Wv, Wo))
    bq, bk, bv, bo = (np.asarray(x, np.float32) for x in (bq, bk, bv, bo))

    if _nc_cache is None:
        _nc_cache = _build()
    in_maps = _make_in_maps(Q, K, V, Wq, bq, Wk, bk, Wv, bv, Wo)
    res = bass_utils.run_bass_kernel_spmd(
        _nc_cache, in_maps, core_ids=list(range(8)), trace=False
    )
    out = np.zeros((2, S, D), np.float32)
    for c in range(8):
        out[c // 4] += res.results[c]["out"].astype(np.float32)
    # v-bias folded out of the device program: attn rows sum to 1, so
    # attn_true @ Wo^T = attn_nobias @ Wo^T + bv @ Wo^T
    out += (bo + bv @ Wo.T)[None, None, :]
    return out
